# revision 15
# baseline (speedup 1.0000x reference)
"""Trainium2 Bass kernel for CausalCoreV5 (complex-weight GNN message passing).

Math: reference does, per step t:
    theta = raw_phase + omega*t ;  c,s = cos(theta), sin(theta)
    Aamp  = A_mask * G_gate * tanh(raw_S) * sigmoid(raw_r)
    out_r = (Aamp*c)@xr - (Aamp*s)@xi ;  out_i = (Aamp*s)@xr + (Aamp*c)@xi
    x'    = tanh([out_r, out_i])

Angle-addition turns the time-varying matrices into two FIXED matrices:
    P = Aamp*cos(raw_phase), Q = Aamp*sin(raw_phase)
    u = P@xr - Q@xi ; v = Q@xr + P@xi
    out_r = cos(wt)*u - sin(wt)*v ; out_i = sin(wt)*u + cos(wt)*v

Each of 8 cores owns 512 output rows. The host stages its five input slices
pre-transposed, partition-major, in bf16 ([128, kt, m] so DMA bursts are 4KB
contiguous); the load phase is pure DMA + elementwise and builds P^T/Q^T in
SBUF as float8e4 scaled by 64. Steady state: 32 DoubleRow fp8 matmuls per
step (adjacent k-tile pairs fused -> 2x PE rate; adjacency also lets step 0
stream behind the load), tanh(PSUM/2048) in bf16, bf16 state AllGather (2KB),
PE-transpose of the gathered state, and the per-step rotation (x32, for fp8
range) folded into fp8 x1/w2 weights. The w2 prep overlaps the P matmuls;
cheap DoubleRow warm matmuls keep the PE busy across the AllGather gap.
"""

import os
import sys

import numpy as np

if "/opt/trn_rl_repo" not in sys.path:
    sys.path.insert(0, "/opt/trn_rl_repo")

N = 4096
STEPS = 32
NCORES = 8
ROWS = N // NCORES          # 512 output rows per core
KT = N // 128               # 32 contraction k-tiles of 128
NG = KT // 2                # 16 DoubleRow groups (adjacent k-tile pairs)
N_WARM1 = 44                # warm matmuls bridging the AllGather window
N_WARM2 = 10                # warm matmuls bridging the prep window
CHUNK_KT = 8                # load-chunk k-tiles -> [128, 4096] tiles
SCALE_PQ = 64.0             # fp8 scale on P,Q
SCALE_X = 32.0              # fp8 scale on the rotated state weights
INV_SCALE = 1.0 / (SCALE_PQ * SCALE_X)

_CACHE = {}


def _build_nc():
    import math

    from concourse import bacc, bass, masks, mybir, tile
    from concourse.bass import AP

    f32 = mybir.dt.float32
    bf16 = mybir.dt.bfloat16
    fp8 = mybir.dt.float8e4
    AF = mybir.ActivationFunctionType
    DR = mybir.MatmulPerfMode.DoubleRow
    HALF_PI = math.pi / 2.0

    nc = bacc.Bacc(
        "TRN2",
        target_bir_lowering=False,
        debug=False,
        enable_asserts=True,
        num_devices=NCORES,
    )

    # Register pi/2 as a const AP (used as Sin bias to get cos).
    _hp = nc.alloc_sbuf_tensor("const-halfpi", [128, 1], f32)
    nc.gpsimd.memset(_hp.ap(), HALF_PI)
    nc.const_aps.aps[(f32, HALF_PI)] = _hp.ap()
    nc.all_engine_barrier()

    # xfull comes in TRANSPOSED: [2, N] (xr row, xi row).
    xfull = nc.dram_tensor("xfull", [2, N], f32, kind="ExternalInput")
    # Pre-transposed, partition-major bf16 slices:
    # mt_*[p, kt*512 + m] = raw[512*core + m, kt*128 + p].
    mt_s = nc.dram_tensor("mt_s", [128, KT * 512], bf16, kind="ExternalInput")
    mt_p = nc.dram_tensor("mt_p", [128, KT * 512], bf16, kind="ExternalInput")
    mt_r = nc.dram_tensor("mt_r", [128, KT * 512], bf16, kind="ExternalInput")
    mt_m = nc.dram_tensor("mt_m", [128, KT * 512], bf16, kind="ExternalInput")
    mt_g = nc.dram_tensor("mt_g", [128, KT * 512], bf16, kind="ExternalInput")
    # Per-step rotation scalars 32*cos(wt), 32*sin(wt) broadcast down 128
    # partitions (cols 2t, 2t+1); derived on host from the scalar omega.
    wrot = nc.dram_tensor("wrot", [128, 2 * STEPS], f32, kind="ExternalInput")
    # Each core writes only its own [2, 512] slice per step; host reassembles.
    out = nc.dram_tensor("out", [STEPS + 1, 2, ROWS], bf16, kind="ExternalOutput")

    with tile.TileContext(nc) as tc:
        with (
            tc.tile_pool(name="big", bufs=1) as big,
            tc.tile_pool(name="work", bufs=2) as work,
            tc.tile_pool(name="small", bufs=2) as small,
            tc.tile_pool(name="psA", bufs=2, space="PSUM") as psA,
            tc.tile_pool(name="psB", bufs=2, space="PSUM") as psB,
            tc.tile_pool(name="dram", bufs=2, space="DRAM") as dpool,
        ):
            identf = big.tile([16, 16], f32, name="identf", tag="identf")
            masks.make_identity(nc, identf)
            identb = big.tile([16, 16], bf16, name="identb", tag="identb")
            masks.make_identity(nc, identb)

            # Persistent transposed matrices: PT[k, n], QT[k, n] as 32 k-tiles
            # of [128, 512] side by side -> [128, 32*512] in fp8 (x64 scale).
            pt = big.tile([128, KT * 512], fp8, name="pt", tag="pt")
            qt = big.tile([128, KT * 512], fp8, name="qt", tag="qt")
            ptH = pt.tensor
            qtH = qt.tensor

            # per-step rotation scalars (x32)
            W = 2 * STEPS
            wrs = small.tile([128, W], f32, name="wrs", tag="wrs", bufs=1)
            nc.sync.dma_start(wrs, wrot[0:128, 0:W])
            wrsH = wrs.tensor

            # x1/w2 column layout (DoubleRow plane step must be %16==0):
            # col(kt, c) = 32*(kt&1) + 2*(kt>>1) + c, so the adjacent pair
            # (2k, 2k+1) has planes at cols {2k, 2k+32} (stride 32).
            def prep_x1(t, psx, x1, e):
                """psx [128,64*e] with psx[p, e*(16j+2r+c)] = x[c,(4r+j)*128+p].
                x1[p, col] = rot_t(x)*32 in fp8 (x1e then x1o)."""
                pxH = psx.tensor
                x1H = x1.tensor
                c_t = AP(wrsH, 2 * t, [[W, 128], [1, 1]])
                s_t = AP(wrsH, 2 * t + 1, [[W, 128], [1, 1]])
                xr_ap = AP(pxH, 0, [[64 * e, 128], [16 * e, 4], [2 * e, 8]])
                xi_ap = AP(pxH, e, [[64 * e, 128], [16 * e, 4], [2 * e, 8]])
                tA = small.tile([128, KT], f32, name=f"tA_{t}", tag="tA")
                tB = small.tile([128, KT], f32, name=f"tB_{t}", tag="tB")
                tC = small.tile([128, KT], f32, name=f"tC_{t}", tag="tC")
                tD = small.tile([128, KT], f32, name=f"tD_{t}", tag="tD")
                # tA..tD are kt-ordered: kt = 4r + j for iter dims (j, r)
                t3 = [[KT, 128], [1, 4], [4, 8]]
                # dst iterates kt = 2a+b ascending -> col = 2a + 32b (+base)
                src2 = [[KT, 128], [2, 16], [1, 2]]
                dst2 = [[2 * KT, 128], [2, 16], [32, 2]]
                # x1e chain on DVE; x1o chain on gpsimd (parallel tracks)
                nc.vector.tensor_scalar_mul(AP(tA.tensor, 0, t3), xr_ap, c_t)
                nc.vector.tensor_scalar_mul(AP(tB.tensor, 0, t3), xi_ap, s_t)
                nc.vector.tensor_tensor(
                    AP(x1H, 0, dst2), AP(tA.tensor, 0, src2),
                    AP(tB.tensor, 0, src2), op=mybir.AluOpType.subtract,
                )
                nc.vector.tensor_scalar_mul(AP(tC.tensor, 0, t3), xr_ap, s_t)
                nc.vector.tensor_scalar_mul(AP(tD.tensor, 0, t3), xi_ap, c_t)
                nc.vector.tensor_tensor(
                    AP(x1H, 1, dst2), AP(tC.tensor, 0, src2),
                    AP(tD.tensor, 0, src2), op=mybir.AluOpType.add,
                )

            def prep_w2(t, x1, w2):
                """w2 = [-xi'|xr'] from x1 = [xr'|xi'] (pairwise col swap)."""
                x1H, w2H = x1.tensor, w2.tensor
                flat = [[2 * KT, 128], [2, KT]]
                nc.vector.tensor_scalar_mul(
                    AP(w2H, 0, flat), AP(x1H, 1, flat), -1.0
                )
                nc.vector.tensor_copy(AP(w2H, 1, flat), AP(x1H, 0, flat))

            # ---------------- initial state -> x1/w2 (before load loop so its
            # DMA + transposes run during the load) ------------------------
            xa0 = work.tile([16, 512], f32, name="xa0", tag="xa0")
            nc.sync.dma_start(xa0, AP(xfull, 0, [[512, 8], [N, 2], [1, 512]]))
            x1 = small.tile([128, 2 * KT], fp8, name="x1_0", tag="x1")
            w2 = small.tile([128, 2 * KT], fp8, name="w2_0", tag="w2")
            psx0 = psA.tile([128, 64], f32, name="psx0", tag="psx")
            for j in range(4):
                nc.tensor.transpose(
                    psx0[:, 16 * j:16 * (j + 1)],
                    xa0[:, j * 128:(j + 1) * 128],
                    identf,
                )
            prep_x1(0, psx0, x1, 1)
            prep_w2(0, x1, w2)

            # ---------------- Phase A: load inputs, build P^T, Q^T ----------
            CC = CHUNK_KT * 512  # 2048 columns per chunk
            for c8 in range(KT // CHUNK_KT):  # 8 chunks of 4 k-tiles
                src = [[KT * 512, 128], [1, CC]]
                off = c8 * CC
                s_in = work.tile([128, CC], bf16, name=f"s_{c8}", tag="s_in")
                r_in = work.tile([128, CC], bf16, name=f"r_{c8}", tag="r_in")
                m_in = work.tile([128, CC], bf16, name=f"m_{c8}", tag="m_in")
                g_in = work.tile([128, CC], bf16, name=f"g_{c8}", tag="g_in")
                p_in = work.tile([128, CC], bf16, name=f"p_{c8}", tag="p_in")
                nc.sync.dma_start(s_in, AP(mt_s, off, src))
                nc.sync.dma_start(m_in, AP(mt_m, off, src))
                nc.scalar.dma_start(r_in, AP(mt_r, off, src))
                nc.scalar.dma_start(g_in, AP(mt_g, off, src))
                nc.gpsimd.dma_start(p_in, AP(mt_p, off, src))  # SWDGE: 1 of 5

                cos_t = work.tile([128, CC], bf16, name=f"c_{c8}", tag="cos_t")
                sin_t = work.tile([128, CC], bf16, name=f"n_{c8}", tag="sin_t")

                # sigmoid via tanh keeps ACT on two LUTs; alternate emission
                # order per chunk parity so ACT reloads each LUT once/chunk.
                def _tanh_ops():
                    nc.scalar.activation(s_in, s_in, AF.Tanh)
                    nc.scalar.activation(r_in, r_in, AF.Tanh, scale=0.5)

                def _sin_ops():
                    nc.scalar.activation(cos_t, p_in, AF.Sin, bias=HALF_PI)
                    nc.scalar.activation(sin_t, p_in, AF.Sin)

                if c8 % 2 == 0:
                    _tanh_ops(); _sin_ops()
                else:
                    _sin_ops(); _tanh_ops()

                # r_in <- 64*sigmoid(raw_r) = 32*tanh(raw_r/2) + 32
                nc.vector.tensor_scalar(
                    r_in, r_in, SCALE_X, SCALE_X,
                    op0=mybir.AluOpType.mult, op1=mybir.AluOpType.add,
                )
                nc.vector.tensor_mul(m_in, m_in, g_in)
                nc.gpsimd.tensor_mul(m_in, m_in, s_in)
                nc.vector.tensor_mul(m_in, m_in, r_in)  # 64*Aamp
                pdst = AP(ptH, c8 * CC, [[KT * 512, 128], [1, CC]])
                qdst = AP(qtH, c8 * CC, [[KT * 512, 128], [1, CC]])
                nc.vector.tensor_mul(pdst, cos_t, m_in)   # fp8 cast on write
                nc.vector.tensor_mul(qdst, sin_t, m_in)

            # ---------------- time loop -------------------------------------
            for t in range(STEPS):
                psuv = psB.tile([2, 512], f32, name=f"uv_{t}", tag="uv")
                x1H, w2H = x1.tensor, w2.tensor
                # DoubleRow fuses adjacent k-tiles (2k, 2k+1): weight planes
                # at cols {2k, 2k+32} (stride 32), moving planes adjacent.
                for g in range(NG):
                    nc.tensor.matmul(
                        psuv,
                        AP(x1H, 2 * g, [[2 * KT, 128], [32, 2], [1, 2]]),
                        AP(ptH, g * 1024,
                           [[KT * 512, 128], [512, 2], [1, 512]]),
                        start=(g == 0),
                        stop=False,
                        perf_mode=DR,
                    )
                for g in range(NG):
                    nc.tensor.matmul(
                        psuv,
                        AP(w2H, 2 * g, [[2 * KT, 128], [32, 2], [1, 2]]),
                        AP(qtH, g * 1024,
                           [[KT * 512, 128], [512, 2], [1, 512]]),
                        start=False,
                        stop=(g == NG - 1),
                        perf_mode=DR,
                    )
                xssb = small.tile([2, 512], bf16, name=f"xs_{t}", tag="xssb")
                nc.scalar.activation(xssb, psuv, AF.Tanh, scale=INV_SCALE)
                if t == STEPS - 1:
                    nc.gpsimd.dma_start(
                        AP(out, (t + 1) * 2 * ROWS, [[ROWS, 2], [1, ROWS]]),
                        xssb,
                    )
                    continue

                # bf16 state slice -> DRAM bounce -> AllGather (trigger goes
                # on the gpsimd queue BEFORE the trajectory write)
                agin = dpool.tile([2, 512], bf16, name=f"agin_{t}", tag="agin")
                nc.sync.dma_start(agin, xssb)
                agout = dpool.tile(
                    [NCORES, 2, 512], bf16, name=f"agout_{t}", tag="agout",
                    addr_space="Shared",
                )
                nc.gpsimd.collective_compute(
                    "AllGather",
                    mybir.AluOpType.bypass,
                    replica_groups=[list(range(NCORES))],
                    ins=[agin],
                    outs=[agout],
                )
                # trajectory: own slice only; host reassembles across cores
                nc.gpsimd.dma_start(
                    AP(out, (t + 1) * 2 * ROWS, [[ROWS, 2], [1, ROWS]]), xssb
                )

                # Warm matmuls keep the PE pstate ramped through the gap
                # (a ~8us idle resets the ramp; post-idle matmuls run ~3x
                # slower for the first ~3us). Bridge 1: AllGather window.
                pswm = psB.tile(
                    [2, 512], f32, name=f"warm_{t}", tag="warm", bufs=1
                )
                for dk in range(N_WARM1):
                    g = dk % NG
                    nc.tensor.matmul(
                        pswm,
                        AP(x1H, 2 * g, [[2 * KT, 128], [32, 2], [1, 2]]),
                        AP(ptH, g * 1024,
                           [[KT * 512, 128], [512, 2], [1, 512]]),
                        start=(dk == 0),
                        stop=(dk == N_WARM1 - 1),
                        perf_mode=DR,
                    )

                # gathered bf16 state -> weight layout for step t+1
                agoH = agout.tensor
                xa = work.tile([16, 512], bf16, name=f"xa_{t}", tag="xa")
                nc.sync.dma_start(xa, AP(agoH, 0, [[512, 16], [1, 512]]))
                x1 = small.tile([128, 2 * KT], fp8, name=f"x1_{t+1}", tag="x1")
                w2 = small.tile([128, 2 * KT], fp8, name=f"w2_{t+1}", tag="w2")
                psx = psA.tile([128, 64], bf16, name=f"px_{t}", tag="psxb")
                for j in range(4):
                    nc.tensor.transpose(
                        psx[:, 16 * j:16 * (j + 1)],
                        xa[:, j * 128:(j + 1) * 128],
                        identb,
                    )
                # Bridge 2: prep window (runs on PE while DVE/GPS prep x1/w2)
                for dk in range(N_WARM2):
                    g = dk % NG
                    nc.tensor.matmul(
                        pswm,
                        AP(x1H, 2 * g, [[2 * KT, 128], [32, 2], [1, 2]]),
                        AP(qtH, g * 1024,
                           [[KT * 512, 128], [512, 2], [1, 512]]),
                        start=(dk == 0),
                        stop=(dk == N_WARM2 - 1),
                        perf_mode=DR,
                    )
                prep_x1(t + 1, psx, x1, 1)
                prep_w2(t + 1, x1, w2)

    nc.compile()
    return nc


def _get_nc():
    if "nc" not in _CACHE:
        _CACHE["nc"] = _build_nc()
    return _CACHE["nc"]


def run(inputs, trace=False):
    import ml_dtypes

    from concourse import bass_utils

    nc = _get_nc()
    x = np.asarray(inputs["x"], np.float32)
    xT = np.ascontiguousarray(x.T)  # [2, N]
    om = float(np.asarray(inputs["omega"], np.float32))
    ts = np.arange(STEPS, dtype=np.float32) * np.float32(om)
    c, s = np.cos(ts, dtype=np.float32), np.sin(ts, dtype=np.float32)
    row = np.zeros(2 * STEPS, np.float32)
    row[0::2] = np.float32(SCALE_X) * c
    row[1::2] = np.float32(SCALE_X) * s
    wrot = np.ascontiguousarray(np.broadcast_to(row, (128, 2 * STEPS)))
    mats = {
        "mt_s": np.asarray(inputs["raw_S"], np.float32),
        "mt_p": np.asarray(inputs["raw_phase"], np.float32),
        "mt_r": np.asarray(inputs["raw_r"], np.float32),
        "mt_m": np.asarray(inputs["A_mask"], np.float32),
        "mt_g": np.asarray(inputs["G_gate"], np.float32),
    }
    in_maps = []
    for ci in range(NCORES):
        rows = slice(ci * ROWS, (ci + 1) * ROWS)
        im = {}
        for k, v in mats.items():
            # [512m, 4096k] -> T -> [32kt, 128p, 512m] -> [128, 32*512] bf16
            mt = v[rows].T.reshape(KT, 128, ROWS).transpose(1, 0, 2)
            im[k] = np.ascontiguousarray(
                mt.reshape(128, KT * ROWS).astype(ml_dtypes.bfloat16)
            )
        im["xfull"] = xT
        im["wrot"] = wrot
        in_maps.append(im)
    res = bass_utils.run_bass_kernel_spmd(
        nc, in_maps, core_ids=list(range(NCORES)), trace=trace
    )
    # reassemble: core i owns output columns [512*i, 512*(i+1))
    full = np.empty((STEPS + 1, 2, N), np.float32)
    full[0] = xT
    for i in range(NCORES):
        oi = np.asarray(res.results[i]["out"]).astype(np.float32)
        full[1:, :, i * ROWS:(i + 1) * ROWS] = oi[1:]
    return np.ascontiguousarray(full.transpose(0, 2, 1)), res


def kernel(**inputs):
    full, _ = run(inputs, trace=False)
    return full


# revision 16
# speedup vs baseline: 1.1028x; 1.1028x over previous
"""Trainium2 Bass kernel for CausalCoreV5 (complex-weight GNN message passing).

Math: reference does, per step t:
    theta = raw_phase + omega*t ;  c,s = cos(theta), sin(theta)
    Aamp  = A_mask * G_gate * tanh(raw_S) * sigmoid(raw_r)
    out_r = (Aamp*c)@xr - (Aamp*s)@xi ;  out_i = (Aamp*s)@xr + (Aamp*c)@xi
    x'    = tanh([out_r, out_i])

Angle-addition turns the time-varying matrices into two FIXED matrices:
    P = Aamp*cos(raw_phase), Q = Aamp*sin(raw_phase)
    u = P@xr - Q@xi ; v = Q@xr + P@xi
    out_r = cos(wt)*u - sin(wt)*v ; out_i = sin(wt)*u + cos(wt)*v

Each of 8 cores owns 512 output rows. The host stages its five input slices
pre-transposed, partition-major, in bf16 ([128, kt, m] so DMA bursts are 4KB
contiguous); the load phase is pure DMA + elementwise and builds P^T/Q^T in
SBUF as float8e4 scaled by 64. Steady state: 32 DoubleRow fp8 matmuls per
step (adjacent k-tile pairs fused -> 2x PE rate; adjacency also lets step 0
stream behind the load), tanh(PSUM/2048) in bf16, bf16 state AllGather (2KB),
PE-transpose of the gathered state, and the per-step rotation (x32, for fp8
range) folded into fp8 x1/w2 weights. The w2 prep overlaps the P matmuls;
cheap DoubleRow warm matmuls keep the PE busy across the AllGather gap.
"""

import os
import sys

import numpy as np

if "/opt/trn_rl_repo" not in sys.path:
    sys.path.insert(0, "/opt/trn_rl_repo")

N = 4096
STEPS = 32
NCORES = 8
ROWS = N // NCORES          # 512 output rows per core
KT = N // 128               # 32 contraction k-tiles of 128
NG = KT // 2                # 16 DoubleRow groups (adjacent k-tile pairs)
N_WARM1 = 44                # warm matmuls bridging the AllGather window
N_WARM2 = 14                # warm matmuls bridging the prep window
CHUNK_KT = 8                # load-chunk k-tiles -> [128, 4096] tiles
SCALE_PQ = 64.0             # fp8 scale on P,Q
SCALE_X = 32.0              # fp8 scale on the rotated state weights
INV_SCALE = 1.0 / (SCALE_PQ * SCALE_X)

_CACHE = {}


def _build_nc():
    import math

    from concourse import bacc, bass, masks, mybir, tile
    from concourse.bass import AP

    f32 = mybir.dt.float32
    bf16 = mybir.dt.bfloat16
    fp8 = mybir.dt.float8e4
    AF = mybir.ActivationFunctionType
    DR = mybir.MatmulPerfMode.DoubleRow
    HALF_PI = math.pi / 2.0

    nc = bacc.Bacc(
        "TRN2",
        target_bir_lowering=False,
        debug=False,
        enable_asserts=True,
        num_devices=NCORES,
    )

    # Register pi/2 as a const AP (used as Sin bias to get cos).
    _hp = nc.alloc_sbuf_tensor("const-halfpi", [128, 1], f32)
    nc.gpsimd.memset(_hp.ap(), HALF_PI)
    nc.const_aps.aps[(f32, HALF_PI)] = _hp.ap()
    nc.all_engine_barrier()

    # xfull comes in TRANSPOSED: [2, N] (xr row, xi row).
    xfull = nc.dram_tensor("xfull", [2, N], f32, kind="ExternalInput")
    # Pre-transposed, partition-major bf16 slices:
    # mt_*[p, kt*512 + m] = raw[512*core + m, kt*128 + p].
    mt_s = nc.dram_tensor("mt_s", [128, KT * 512], bf16, kind="ExternalInput")
    mt_p = nc.dram_tensor("mt_p", [128, KT * 512], bf16, kind="ExternalInput")
    mt_r = nc.dram_tensor("mt_r", [128, KT * 512], bf16, kind="ExternalInput")
    # mask/gate are exact in fp8 (0/1-valued); halves their DMA traffic
    mt_m = nc.dram_tensor("mt_m", [128, KT * 512], fp8, kind="ExternalInput")
    mt_g = nc.dram_tensor("mt_g", [128, KT * 512], fp8, kind="ExternalInput")
    # Per-step rotation scalars 32*cos(wt), 32*sin(wt) broadcast down 128
    # partitions (cols 2t, 2t+1); derived on host from the scalar omega.
    wrot = nc.dram_tensor("wrot", [128, 2 * STEPS], f32, kind="ExternalInput")
    # Each core writes only its own [2, 512] slice per step; host reassembles.
    out = nc.dram_tensor("out", [STEPS + 1, 2, ROWS], bf16, kind="ExternalOutput")

    with tile.TileContext(nc) as tc:
        with (
            tc.tile_pool(name="big", bufs=1) as big,
            tc.tile_pool(name="work", bufs=2) as work,
            tc.tile_pool(name="small", bufs=2) as small,
            tc.tile_pool(name="psA", bufs=2, space="PSUM") as psA,
            tc.tile_pool(name="psB", bufs=2, space="PSUM") as psB,
            tc.tile_pool(name="dram", bufs=2, space="DRAM") as dpool,
        ):
            identf = big.tile([16, 16], f32, name="identf", tag="identf")
            masks.make_identity(nc, identf)
            identb = big.tile([16, 16], bf16, name="identb", tag="identb")
            masks.make_identity(nc, identb)

            # Persistent transposed matrices: PT[k, n], QT[k, n] as 32 k-tiles
            # of [128, 512] side by side -> [128, 32*512] in fp8 (x64 scale).
            pt = big.tile([128, KT * 512], fp8, name="pt", tag="pt")
            qt = big.tile([128, KT * 512], fp8, name="qt", tag="qt")
            ptH = pt.tensor
            qtH = qt.tensor

            # per-step rotation scalars (x32)
            W = 2 * STEPS
            wrs = small.tile([128, W], f32, name="wrs", tag="wrs", bufs=1)
            nc.sync.dma_start(wrs, wrot[0:128, 0:W])
            wrsH = wrs.tensor

            # x1/w2 column layout (DoubleRow plane step must be %16==0):
            # col(kt, c) = 32*(kt&1) + 2*(kt>>1) + c, so the adjacent pair
            # (2k, 2k+1) has planes at cols {2k, 2k+32} (stride 32).
            def prep_x1(t, psx, x1, e):
                """psx [128,64*e] with psx[p, e*(16j+2r+c)] = x[c,(4r+j)*128+p].
                x1[p, col] = rot_t(x)*32 in fp8 (x1e then x1o)."""
                pxH = psx.tensor
                x1H = x1.tensor
                c_t = AP(wrsH, 2 * t, [[W, 128], [1, 1]])
                s_t = AP(wrsH, 2 * t + 1, [[W, 128], [1, 1]])
                xr_ap = AP(pxH, 0, [[64 * e, 128], [16 * e, 4], [2 * e, 8]])
                xi_ap = AP(pxH, e, [[64 * e, 128], [16 * e, 4], [2 * e, 8]])
                tA = small.tile([128, KT], f32, name=f"tA_{t}", tag="tA")
                tB = small.tile([128, KT], f32, name=f"tB_{t}", tag="tB")
                tC = small.tile([128, KT], f32, name=f"tC_{t}", tag="tC")
                tD = small.tile([128, KT], f32, name=f"tD_{t}", tag="tD")
                # tA..tD are kt-ordered: kt = 4r + j for iter dims (j, r)
                t3 = [[KT, 128], [1, 4], [4, 8]]
                # dst iterates kt = 2a+b ascending -> col = 2a + 32b (+base)
                src2 = [[KT, 128], [2, 16], [1, 2]]
                dst2 = [[2 * KT, 128], [2, 16], [32, 2]]
                # x1e chain on DVE; x1o chain on gpsimd (parallel tracks)
                nc.vector.tensor_scalar_mul(AP(tA.tensor, 0, t3), xr_ap, c_t)
                nc.vector.tensor_scalar_mul(AP(tB.tensor, 0, t3), xi_ap, s_t)
                nc.vector.tensor_tensor(
                    AP(x1H, 0, dst2), AP(tA.tensor, 0, src2),
                    AP(tB.tensor, 0, src2), op=mybir.AluOpType.subtract,
                )
                nc.vector.tensor_scalar_mul(AP(tC.tensor, 0, t3), xr_ap, s_t)
                nc.vector.tensor_scalar_mul(AP(tD.tensor, 0, t3), xi_ap, c_t)
                nc.vector.tensor_tensor(
                    AP(x1H, 1, dst2), AP(tC.tensor, 0, src2),
                    AP(tD.tensor, 0, src2), op=mybir.AluOpType.add,
                )

            def prep_w2(t, x1, w2):
                """w2 = [-xi'|xr'] from x1 = [xr'|xi'] (pairwise col swap)."""
                x1H, w2H = x1.tensor, w2.tensor
                flat = [[2 * KT, 128], [2, KT]]
                nc.vector.tensor_scalar_mul(
                    AP(w2H, 0, flat), AP(x1H, 1, flat), -1.0
                )
                nc.vector.tensor_copy(AP(w2H, 1, flat), AP(x1H, 0, flat))

            # ---------------- initial state -> x1/w2 (before load loop so its
            # DMA + transposes run during the load) ------------------------
            xa0 = work.tile([16, 512], f32, name="xa0", tag="xa0")
            nc.sync.dma_start(xa0, AP(xfull, 0, [[512, 8], [N, 2], [1, 512]]))
            x1 = small.tile([128, 2 * KT], fp8, name="x1_0", tag="x1")
            w2 = small.tile([128, 2 * KT], fp8, name="w2_0", tag="w2")
            psx0 = psA.tile([128, 64], f32, name="psx0", tag="psx")
            for j in range(4):
                nc.tensor.transpose(
                    psx0[:, 16 * j:16 * (j + 1)],
                    xa0[:, j * 128:(j + 1) * 128],
                    identf,
                )
            prep_x1(0, psx0, x1, 1)
            prep_w2(0, x1, w2)

            # ---------------- Phase A: load inputs, build P^T, Q^T ----------
            CC = CHUNK_KT * 512  # 2048 columns per chunk
            for c8 in range(KT // CHUNK_KT):  # 8 chunks of 4 k-tiles
                src = [[KT * 512, 128], [1, CC]]
                off = c8 * CC
                s_in = work.tile([128, CC], bf16, name=f"s_{c8}", tag="s_in")
                r_in = work.tile([128, CC], bf16, name=f"r_{c8}", tag="r_in")
                m_in = work.tile([128, CC], fp8, name=f"m_{c8}", tag="m_in")
                g_in = work.tile([128, CC], fp8, name=f"g_{c8}", tag="g_in")
                p_in = work.tile([128, CC], bf16, name=f"p_{c8}", tag="p_in")
                nc.sync.dma_start(s_in, AP(mt_s, off, src))
                nc.sync.dma_start(g_in, AP(mt_g, off, src))
                nc.scalar.dma_start(p_in, AP(mt_p, off, src))
                nc.scalar.dma_start(r_in, AP(mt_r, off, src))
                nc.gpsimd.dma_start(m_in, AP(mt_m, off, src))  # SWDGE: smallest

                cos_t = work.tile([128, CC], bf16, name=f"c_{c8}", tag="cos_t")
                sin_t = work.tile([128, CC], bf16, name=f"n_{c8}", tag="sin_t")

                # sigmoid via tanh keeps ACT on two LUTs; alternate emission
                # order per chunk parity so ACT reloads each LUT once/chunk.
                def _tanh_ops():
                    nc.scalar.activation(s_in, s_in, AF.Tanh)
                    nc.scalar.activation(r_in, r_in, AF.Tanh, scale=0.5)

                def _sin_ops():
                    nc.scalar.activation(cos_t, p_in, AF.Sin, bias=HALF_PI)
                    nc.scalar.activation(sin_t, p_in, AF.Sin)

                if c8 % 2 == 0:
                    _tanh_ops(); _sin_ops()
                else:
                    _sin_ops(); _tanh_ops()

                # r_in <- 64*sigmoid(raw_r) = 32*tanh(raw_r/2) + 32
                nc.vector.tensor_scalar(
                    r_in, r_in, SCALE_X, SCALE_X,
                    op0=mybir.AluOpType.mult, op1=mybir.AluOpType.add,
                )
                mb = work.tile([128, CC], bf16, name=f"mb_{c8}", tag="mb")
                nc.vector.tensor_mul(mb, m_in, g_in)
                nc.gpsimd.tensor_mul(mb, mb, s_in)
                nc.vector.tensor_mul(mb, mb, r_in)  # 64*Aamp
                pdst = AP(ptH, c8 * CC, [[KT * 512, 128], [1, CC]])
                qdst = AP(qtH, c8 * CC, [[KT * 512, 128], [1, CC]])
                nc.vector.tensor_mul(pdst, cos_t, mb)   # fp8 cast on write
                nc.vector.tensor_mul(qdst, sin_t, mb)

            # ---------------- time loop -------------------------------------
            for t in range(STEPS):
                psuv = psB.tile([2, 512], f32, name=f"uv_{t}", tag="uv")
                x1H, w2H = x1.tensor, w2.tensor
                # DoubleRow fuses adjacent k-tiles (2k, 2k+1): weight planes
                # at cols {2k, 2k+32} (stride 32), moving planes adjacent.
                for g in range(NG):
                    nc.tensor.matmul(
                        psuv,
                        AP(x1H, 2 * g, [[2 * KT, 128], [32, 2], [1, 2]]),
                        AP(ptH, g * 1024,
                           [[KT * 512, 128], [512, 2], [1, 512]]),
                        start=(g == 0),
                        stop=False,
                        perf_mode=DR,
                    )
                for g in range(NG):
                    nc.tensor.matmul(
                        psuv,
                        AP(w2H, 2 * g, [[2 * KT, 128], [32, 2], [1, 2]]),
                        AP(qtH, g * 1024,
                           [[KT * 512, 128], [512, 2], [1, 512]]),
                        start=False,
                        stop=(g == NG - 1),
                        perf_mode=DR,
                    )
                xssb = small.tile([2, 512], bf16, name=f"xs_{t}", tag="xssb")
                nc.scalar.activation(xssb, psuv, AF.Tanh, scale=INV_SCALE)
                if t == STEPS - 1:
                    nc.gpsimd.dma_start(
                        AP(out, (t + 1) * 2 * ROWS, [[ROWS, 2], [1, ROWS]]),
                        xssb,
                    )
                    continue

                # bf16 state slice -> DRAM bounce -> AllGather (trigger goes
                # on the gpsimd queue BEFORE the trajectory write)
                agin = dpool.tile([2, 512], bf16, name=f"agin_{t}", tag="agin")
                nc.sync.dma_start(agin, xssb)
                agout = dpool.tile(
                    [NCORES, 2, 512], bf16, name=f"agout_{t}", tag="agout",
                    addr_space="Shared",
                )
                nc.gpsimd.collective_compute(
                    "AllGather",
                    mybir.AluOpType.bypass,
                    replica_groups=[list(range(NCORES))],
                    ins=[agin],
                    outs=[agout],
                )
                # trajectory: own slice only; host reassembles across
                # cores. On the sync queue: SWDGE would contend with the
                # in-flight collective.
                nc.sync.dma_start(
                    AP(out, (t + 1) * 2 * ROWS, [[ROWS, 2], [1, ROWS]]), xssb
                )

                # Warm matmuls keep the PE pstate ramped through the gap
                # (a ~8us idle resets the ramp; post-idle matmuls run ~3x
                # slower for the first ~3us). Bridge 1: AllGather window.
                pswm = psB.tile(
                    [2, 512], f32, name=f"warm_{t}", tag="warm", bufs=1
                )
                for dk in range(N_WARM1):
                    g = dk % NG
                    nc.tensor.matmul(
                        pswm,
                        AP(x1H, 2 * g, [[2 * KT, 128], [32, 2], [1, 2]]),
                        AP(ptH, g * 1024,
                           [[KT * 512, 128], [512, 2], [1, 512]]),
                        start=(dk == 0),
                        stop=(dk == N_WARM1 - 1),
                        perf_mode=DR,
                    )

                # gathered bf16 state -> weight layout for step t+1
                agoH = agout.tensor
                xa = work.tile([16, 512], bf16, name=f"xa_{t}", tag="xa")
                nc.sync.dma_start(xa, AP(agoH, 0, [[512, 16], [1, 512]]))
                x1 = small.tile([128, 2 * KT], fp8, name=f"x1_{t+1}", tag="x1")
                w2 = small.tile([128, 2 * KT], fp8, name=f"w2_{t+1}", tag="w2")
                psx = psA.tile([128, 64], bf16, name=f"px_{t}", tag="psxb")
                for j in range(4):
                    nc.tensor.transpose(
                        psx[:, 16 * j:16 * (j + 1)],
                        xa[:, j * 128:(j + 1) * 128],
                        identb,
                    )
                # Bridge 2: prep window (runs on PE while DVE/GPS prep x1/w2)
                for dk in range(N_WARM2):
                    g = dk % NG
                    nc.tensor.matmul(
                        pswm,
                        AP(x1H, 2 * g, [[2 * KT, 128], [32, 2], [1, 2]]),
                        AP(qtH, g * 1024,
                           [[KT * 512, 128], [512, 2], [1, 512]]),
                        start=(dk == 0),
                        stop=(dk == N_WARM2 - 1),
                        perf_mode=DR,
                    )
                prep_x1(t + 1, psx, x1, 1)
                prep_w2(t + 1, x1, w2)

    nc.compile()
    return nc


def _get_nc():
    if "nc" not in _CACHE:
        _CACHE["nc"] = _build_nc()
    return _CACHE["nc"]


def run(inputs, trace=False):
    import ml_dtypes

    from concourse import bass_utils

    nc = _get_nc()
    x = np.asarray(inputs["x"], np.float32)
    xT = np.ascontiguousarray(x.T)  # [2, N]
    om = float(np.asarray(inputs["omega"], np.float32))
    ts = np.arange(STEPS, dtype=np.float32) * np.float32(om)
    c, s = np.cos(ts, dtype=np.float32), np.sin(ts, dtype=np.float32)
    row = np.zeros(2 * STEPS, np.float32)
    row[0::2] = np.float32(SCALE_X) * c
    row[1::2] = np.float32(SCALE_X) * s
    wrot = np.ascontiguousarray(np.broadcast_to(row, (128, 2 * STEPS)))
    mats = {
        "mt_s": np.asarray(inputs["raw_S"], np.float32),
        "mt_p": np.asarray(inputs["raw_phase"], np.float32),
        "mt_r": np.asarray(inputs["raw_r"], np.float32),
        "mt_m": np.asarray(inputs["A_mask"], np.float32),
        "mt_g": np.asarray(inputs["G_gate"], np.float32),
    }
    in_maps = []
    for ci in range(NCORES):
        rows = slice(ci * ROWS, (ci + 1) * ROWS)
        im = {}
        for k, v in mats.items():
            # [512m, 4096k] -> T -> [32kt, 128p, 512m] -> [128, 32*512]
            dt = ml_dtypes.float8_e4m3 if k in ("mt_m", "mt_g") else \
                ml_dtypes.bfloat16
            mt = v[rows].T.reshape(KT, 128, ROWS).transpose(1, 0, 2)
            im[k] = np.ascontiguousarray(
                mt.reshape(128, KT * ROWS).astype(dt)
            )
        im["xfull"] = xT
        im["wrot"] = wrot
        in_maps.append(im)
    res = bass_utils.run_bass_kernel_spmd(
        nc, in_maps, core_ids=list(range(NCORES)), trace=trace
    )
    # reassemble: core i owns output columns [512*i, 512*(i+1))
    full = np.empty((STEPS + 1, 2, N), np.float32)
    full[0] = xT
    for i in range(NCORES):
        oi = np.asarray(res.results[i]["out"]).astype(np.float32)
        full[1:, :, i * ROWS:(i + 1) * ROWS] = oi[1:]
    return np.ascontiguousarray(full.transpose(0, 2, 1)), res


def kernel(**inputs):
    full, _ = run(inputs, trace=False)
    return full


# revision 20
# speedup vs baseline: 1.1180x; 1.0137x over previous
"""Trainium2 Bass kernel for CausalCoreV5 (complex-weight GNN message passing).

Math: reference does, per step t:
    theta = raw_phase + omega*t ;  c,s = cos(theta), sin(theta)
    Aamp  = A_mask * G_gate * tanh(raw_S) * sigmoid(raw_r)
    out_r = (Aamp*c)@xr - (Aamp*s)@xi ;  out_i = (Aamp*s)@xr + (Aamp*c)@xi
    x'    = tanh([out_r, out_i])

Angle-addition turns the time-varying matrices into two FIXED matrices:
    P = Aamp*cos(raw_phase), Q = Aamp*sin(raw_phase)
    u = P@xr - Q@xi ; v = Q@xr + P@xi
    out_r = cos(wt)*u - sin(wt)*v ; out_i = sin(wt)*u + cos(wt)*v

Each of 8 cores owns 512 output rows. The host stages its five input slices
pre-transposed, partition-major, in bf16 ([128, kt, m] so DMA bursts are 4KB
contiguous); the load phase is pure DMA + elementwise and builds P^T/Q^T in
SBUF as float8e4 scaled by 64. Steady state: 32 DoubleRow fp8 matmuls per
step (adjacent k-tile pairs fused -> 2x PE rate; adjacency also lets step 0
stream behind the load), tanh(PSUM/2048) in bf16, bf16 state AllGather (2KB),
PE-transpose of the gathered state, and the per-step rotation (x32, for fp8
range) folded into fp8 x1/w2 weights. The w2 prep overlaps the P matmuls;
cheap DoubleRow warm matmuls keep the PE busy across the AllGather gap.
"""

import os
import sys

import numpy as np

if "/opt/trn_rl_repo" not in sys.path:
    sys.path.insert(0, "/opt/trn_rl_repo")

N = 4096
STEPS = 32
NCORES = 8
ROWS = N // NCORES          # 512 output rows per core
KT = N // 128               # 32 contraction k-tiles of 128
NG = KT // 2                # 16 DoubleRow groups (adjacent k-tile pairs)
N_WARM1 = 52                # warm matmuls bridging the AllGather window
N_WARM2 = 10                # warm matmuls bridging the prep window
CHUNK_KT = 8                # load-chunk k-tiles -> [128, 4096] tiles
SCALE_PQ = 64.0             # fp8 scale on P,Q
SCALE_X = 32.0              # fp8 scale on the rotated state weights
INV_SCALE = 1.0 / (SCALE_PQ * SCALE_X)

_CACHE = {}


def _build_nc():
    import math

    from concourse import bacc, bass, masks, mybir, tile
    from concourse.bass import AP

    f32 = mybir.dt.float32
    bf16 = mybir.dt.bfloat16
    fp8 = mybir.dt.float8e4
    AF = mybir.ActivationFunctionType
    DR = mybir.MatmulPerfMode.DoubleRow
    HALF_PI = math.pi / 2.0

    nc = bacc.Bacc(
        "TRN2",
        target_bir_lowering=False,
        debug=False,
        enable_asserts=True,
        num_devices=NCORES,
    )

    # Register pi/2 as a const AP (used as Sin bias to get cos).
    _hp = nc.alloc_sbuf_tensor("const-halfpi", [128, 1], f32)
    nc.gpsimd.memset(_hp.ap(), HALF_PI)
    nc.const_aps.aps[(f32, HALF_PI)] = _hp.ap()
    nc.all_engine_barrier()

    # xfull comes in TRANSPOSED: [2, N] (xr row, xi row).
    xfull = nc.dram_tensor("xfull", [2, N], f32, kind="ExternalInput")
    # Pre-transposed, partition-major bf16 slices:
    # mt_*[p, kt*512 + m] = raw[512*core + m, kt*128 + p].
    mt_s = nc.dram_tensor("mt_s", [128, KT * 512], bf16, kind="ExternalInput")
    mt_p = nc.dram_tensor("mt_p", [128, KT * 512], bf16, kind="ExternalInput")
    mt_r = nc.dram_tensor("mt_r", [128, KT * 512], bf16, kind="ExternalInput")
    # mask/gate are exact in fp8 (0/1-valued); halves their DMA traffic
    mt_m = nc.dram_tensor("mt_m", [128, KT * 512], fp8, kind="ExternalInput")
    mt_g = nc.dram_tensor("mt_g", [128, KT * 512], fp8, kind="ExternalInput")
    # Per-step rotation scalars 32*cos(wt), 32*sin(wt) broadcast down 128
    # partitions (cols 2t, 2t+1); derived on host from the scalar omega.
    wrot = nc.dram_tensor("wrot", [128, 3 * STEPS], f32, kind="ExternalInput")
    # rotation building blocks: I16 and J (J[2r,2r+1]=1, J[2r+1,2r]=-1)
    rotc = nc.dram_tensor("rotc", [128, 32], bf16, kind="ExternalInput")
    # Each core writes only its own [2, 512] slice per step; host reassembles.
    out = nc.dram_tensor("out", [STEPS + 1, 2, ROWS], bf16, kind="ExternalOutput")

    with tile.TileContext(nc) as tc:
        with (
            tc.tile_pool(name="big", bufs=1) as big,
            tc.tile_pool(name="work", bufs=2) as work,
            tc.tile_pool(name="small", bufs=2) as small,
            tc.tile_pool(name="psA", bufs=2, space="PSUM") as psA,
            tc.tile_pool(name="psB", bufs=2, space="PSUM") as psB,
            tc.tile_pool(name="dram", bufs=2, space="DRAM") as dpool,
        ):
            rotIJ = big.tile([128, 32], bf16, name="rotIJ", tag="rotIJ")
            nc.sync.dma_start(rotIJ, rotc[0:128, 0:32])
            rotH = rotIJ.tensor
            # persistent zero-padded state buffers: AG lands rows 0-15, rows
            # 16-127 stay zero so the 128-contraction rotation matmuls are
            # safe (sub-128-partition matmuls hang the PE).
            xaA = big.tile([128, 512], bf16, name="xaA", tag="xaA")
            xaB = big.tile([128, 512], bf16, name="xaB", tag="xaB")
            nc.gpsimd.memset(xaA, 0.0)
            nc.gpsimd.memset(xaB, 0.0)

            # Persistent transposed matrices: PT[k, n], QT[k, n] as 32 k-tiles
            # of [128, 512] side by side -> [128, 32*512] in fp8 (x64 scale).
            pt = big.tile([128, KT * 512], fp8, name="pt", tag="pt")
            qt = big.tile([128, KT * 512], fp8, name="qt", tag="qt")
            ptH = pt.tensor
            qtH = qt.tensor

            # per-step rotation scalars (32c, 32s, -32s)
            W = 3 * STEPS
            wrs = small.tile([128, W], f32, name="wrs", tag="wrs", bufs=1)
            nc.sync.dma_start(wrs, wrot[0:128, 0:W])
            wrsH = wrs.tensor

            # x1/w2 column layout (DoubleRow plane step must be %16==0):
            # col(kt, c) = 32*(kt&1) + 2*(kt>>1) + c, so the adjacent pair
            # (2k, 2k+1) has planes at cols {2k, 2k+32} (stride 32).
            I_ap = AP(rotH, 0, [[32, 128], [1, 16]])
            J_ap = AP(rotH, 16, [[32, 128], [1, 16]])

            def build_rots(t):
                """ROT1 = 32*(c*I + s*J), ROT2 = 32*(-s*I + c*J): [16,16] bf16
                block-rotation matrices; runs on DVE off the critical path."""
                c_t = AP(wrsH, 3 * t, [[W, 128], [1, 1]])
                s_t = AP(wrsH, 3 * t + 1, [[W, 128], [1, 1]])
                ns_t = AP(wrsH, 3 * t + 2, [[W, 128], [1, 1]])
                r1 = small.tile([128, 16], bf16, name=f"r1_{t}", tag="r1")
                r2 = small.tile([128, 16], bf16, name=f"r2_{t}", tag="r2")
                tj = small.tile([128, 16], bf16, name=f"tj_{t}", tag="tj")
                nc.vector.tensor_scalar_mul(r1, I_ap, c_t)
                nc.vector.tensor_scalar_mul(tj, J_ap, s_t)
                nc.vector.tensor_tensor(r1, r1, tj, op=mybir.AluOpType.add)
                nc.vector.tensor_scalar_mul(r2, I_ap, ns_t)
                nc.vector.tensor_scalar_mul(tj, J_ap, c_t)
                nc.vector.tensor_tensor(r2, r2, tj, op=mybir.AluOpType.add)
                return r1, r2

            def rot_mms(xa_t, r1, r2, t):
                """8 mini-matmuls: psx1[p,16j+cc'] = sum_cc xa[cc,j*128+p] *
                ROT[cc,cc'] -> rotated state in transposed layout (rows of
                xa_t/ROT beyond 15 are zero padding)."""
                psx1 = psA.tile([128, 64], f32, name=f"p1_{t}", tag="psx1")
                psx2 = psA.tile([128, 64], f32, name=f"p2_{t}", tag="psx2")
                for j in range(4):
                    nc.tensor.matmul(
                        psx1[:, 16 * j:16 * (j + 1)],
                        xa_t[:, j * 128:(j + 1) * 128], r1,
                        start=True, stop=True,
                    )
                for j in range(4):
                    nc.tensor.matmul(
                        psx2[:, 16 * j:16 * (j + 1)],
                        xa_t[:, j * 128:(j + 1) * 128], r2,
                        start=True, stop=True,
                    )
                return psx1, psx2

            def copy_weights(psx1, psx2, x1, w2):
                """psx col (32j1+16j0+2r+c) -> weight col (32j0+2j1+4r+c);
                two strided f32->fp8 copies per weight tile."""
                for dstT, srcT in ((x1, psx1), (w2, psx2)):
                    dH, sH = dstT.tensor, srcT.tensor
                    for j0 in range(2):
                        nc.vector.tensor_copy(
                            AP(dH, 32 * j0, [[64, 128], [2, 2], [4, 8], [1, 2]]),
                            AP(sH, 16 * j0, [[64, 128], [32, 2], [2, 8], [1, 2]]),
                        )

            # ---------------- initial state -> x1/w2 (before load loop so its
            # DMA + transposes run during the load) ------------------------
            xa0 = work.tile([16, 512], f32, name="xa0", tag="xa0")
            nc.sync.dma_start(xa0, AP(xfull, 0, [[512, 8], [N, 2], [1, 512]]))
            nc.vector.tensor_copy(xaA[0:16, :], xa0)
            x1 = small.tile([128, 2 * KT], fp8, name="x1_0", tag="x1")
            w2 = small.tile([128, 2 * KT], fp8, name="w2_0", tag="w2")
            r1, r2 = build_rots(0)
            psx1, psx2 = rot_mms(xaA, r1, r2, 0)
            copy_weights(psx1, psx2, x1, w2)

            # ---------------- Phase A: load inputs, build P^T, Q^T ----------
            CC = CHUNK_KT * 512  # 2048 columns per chunk
            for c8 in range(KT // CHUNK_KT):  # 8 chunks of 4 k-tiles
                src = [[KT * 512, 128], [1, CC]]
                off = c8 * CC
                s_in = work.tile([128, CC], bf16, name=f"s_{c8}", tag="s_in")
                r_in = work.tile([128, CC], bf16, name=f"r_{c8}", tag="r_in")
                m_in = work.tile([128, CC], fp8, name=f"m_{c8}", tag="m_in")
                g_in = work.tile([128, CC], fp8, name=f"g_{c8}", tag="g_in")
                p_in = work.tile([128, CC], bf16, name=f"p_{c8}", tag="p_in")
                nc.scalar.dma_start(s_in, AP(mt_s, off, src))
                nc.scalar.dma_start(p_in, AP(mt_p, off, src))
                nc.scalar.dma_start(g_in, AP(mt_g, off, src))
                nc.sync.dma_start(r_in, AP(mt_r, off, src))
                nc.gpsimd.dma_start(m_in, AP(mt_m, off, src))  # SWDGE: smallest

                cos_t = work.tile([128, CC], bf16, name=f"c_{c8}", tag="cos_t")
                sin_t = work.tile([128, CC], bf16, name=f"n_{c8}", tag="sin_t")

                # sigmoid via tanh keeps ACT on two LUTs; alternate emission
                # order per chunk parity so ACT reloads each LUT once/chunk.
                def _tanh_ops():
                    nc.scalar.activation(s_in, s_in, AF.Tanh)
                    nc.scalar.activation(r_in, r_in, AF.Tanh, scale=0.5)

                def _sin_ops():
                    nc.scalar.activation(cos_t, p_in, AF.Sin, bias=HALF_PI)
                    nc.scalar.activation(sin_t, p_in, AF.Sin)

                if c8 % 2 == 0:
                    _tanh_ops(); _sin_ops()
                else:
                    _sin_ops(); _tanh_ops()

                # r_in <- 64*sigmoid(raw_r) = 32*tanh(raw_r/2) + 32
                nc.vector.tensor_scalar(
                    r_in, r_in, SCALE_X, SCALE_X,
                    op0=mybir.AluOpType.mult, op1=mybir.AluOpType.add,
                )
                mb = work.tile([128, CC], bf16, name=f"mb_{c8}", tag="mb")
                nc.vector.tensor_mul(mb, m_in, g_in)
                nc.gpsimd.tensor_mul(mb, mb, s_in)
                nc.vector.tensor_mul(mb, mb, r_in)  # 64*Aamp
                pdst = AP(ptH, c8 * CC, [[KT * 512, 128], [1, CC]])
                qdst = AP(qtH, c8 * CC, [[KT * 512, 128], [1, CC]])
                nc.vector.tensor_mul(pdst, cos_t, mb)   # fp8 cast on write
                nc.vector.tensor_mul(qdst, sin_t, mb)

            # ---------------- time loop -------------------------------------
            for t in range(STEPS):
                psuv = psB.tile([2, 512], f32, name=f"uv_{t}", tag="uv")
                x1H, w2H = x1.tensor, w2.tensor
                # DoubleRow fuses adjacent k-tiles (2k, 2k+1): weight planes
                # at cols {2k, 2k+32} (stride 32), moving planes adjacent.
                for g in range(NG):
                    nc.tensor.matmul(
                        psuv,
                        AP(x1H, 2 * g, [[2 * KT, 128], [32, 2], [1, 2]]),
                        AP(ptH, g * 1024,
                           [[KT * 512, 128], [512, 2], [1, 512]]),
                        start=(g == 0),
                        stop=False,
                        perf_mode=DR,
                    )
                for g in range(NG):
                    nc.tensor.matmul(
                        psuv,
                        AP(w2H, 2 * g, [[2 * KT, 128], [32, 2], [1, 2]]),
                        AP(qtH, g * 1024,
                           [[KT * 512, 128], [512, 2], [1, 512]]),
                        start=False,
                        stop=(g == NG - 1),
                        perf_mode=DR,
                    )
                xssb = small.tile([2, 512], bf16, name=f"xs_{t}", tag="xssb")
                nc.scalar.activation(xssb, psuv, AF.Tanh, scale=INV_SCALE)
                if t == STEPS - 1:
                    nc.gpsimd.dma_start(
                        AP(out, (t + 1) * 2 * ROWS, [[ROWS, 2], [1, ROWS]]),
                        xssb,
                    )
                    continue

                # bf16 state slice -> DRAM bounce -> AllGather (trigger goes
                # on the gpsimd queue BEFORE the trajectory write)
                agin = dpool.tile([2, 512], bf16, name=f"agin_{t}", tag="agin")
                nc.sync.dma_start(agin, xssb)
                agout = dpool.tile(
                    [NCORES, 2, 512], bf16, name=f"agout_{t}", tag="agout",
                    addr_space="Shared",
                )
                nc.gpsimd.collective_compute(
                    "AllGather",
                    mybir.AluOpType.bypass,
                    replica_groups=[list(range(NCORES))],
                    ins=[agin],
                    outs=[agout],
                )
                # trajectory: own slice only; host reassembles across
                # cores. On the sync queue: SWDGE would contend with the
                # in-flight collective.
                nc.sync.dma_start(
                    AP(out, (t + 1) * 2 * ROWS, [[ROWS, 2], [1, ROWS]]), xssb
                )

                # Warm matmuls keep the PE pstate ramped through the gap
                # (a ~8us idle resets the ramp; post-idle matmuls run ~3x
                # slower for the first ~3us). Bridge 1: AllGather window.
                pswm = psB.tile(
                    [2, 512], f32, name=f"warm_{t}", tag="warm", bufs=1
                )
                for dk in range(N_WARM1):
                    g = dk % NG
                    nc.tensor.matmul(
                        pswm,
                        AP(x1H, 2 * g, [[2 * KT, 128], [32, 2], [1, 2]]),
                        AP(ptH, g * 1024,
                           [[KT * 512, 128], [512, 2], [1, 512]]),
                        start=(dk == 0),
                        stop=(dk == N_WARM1 - 1),
                        perf_mode=DR,
                    )

                # gathered bf16 state -> rotated weight layout for t+1.
                # ROT matrices for t+1 build on DVE during the AG wait.
                r1, r2 = build_rots(t + 1)
                agoH = agout.tensor
                xa = xaB if (t + 1) % 2 else xaA
                nc.sync.dma_start(
                    xa[0:16, :], AP(agoH, 0, [[512, 16], [1, 512]])
                )
                x1 = small.tile([128, 2 * KT], fp8, name=f"x1_{t+1}", tag="x1")
                w2 = small.tile([128, 2 * KT], fp8, name=f"w2_{t+1}", tag="w2")
                psx1, psx2 = rot_mms(xa, r1, r2, t + 1)
                # Bridge 2: copy window (PE warm while DVE writes x1/w2)
                for dk in range(N_WARM2):
                    g = dk % NG
                    nc.tensor.matmul(
                        pswm,
                        AP(x1H, 2 * g, [[2 * KT, 128], [32, 2], [1, 2]]),
                        AP(qtH, g * 1024,
                           [[KT * 512, 128], [512, 2], [1, 512]]),
                        start=(dk == 0),
                        stop=(dk == N_WARM2 - 1),
                        perf_mode=DR,
                    )
                copy_weights(psx1, psx2, x1, w2)

    nc.compile()
    return nc


def _get_nc():
    if "nc" not in _CACHE:
        _CACHE["nc"] = _build_nc()
    return _CACHE["nc"]


def run(inputs, trace=False):
    import ml_dtypes

    from concourse import bass_utils

    nc = _get_nc()
    x = np.asarray(inputs["x"], np.float32)
    xT = np.ascontiguousarray(x.T)  # [2, N]
    om = float(np.asarray(inputs["omega"], np.float32))
    ts = np.arange(STEPS, dtype=np.float32) * np.float32(om)
    c, s = np.cos(ts, dtype=np.float32), np.sin(ts, dtype=np.float32)
    row = np.zeros(3 * STEPS, np.float32)
    row[0::3] = np.float32(SCALE_X) * c
    row[1::3] = np.float32(SCALE_X) * s
    row[2::3] = np.float32(-SCALE_X) * s
    wrot = np.ascontiguousarray(np.broadcast_to(row, (128, 3 * STEPS)))
    eye = np.eye(16, dtype=np.float32)
    jmat = np.zeros((16, 16), np.float32)
    for rr in range(8):
        jmat[2 * rr, 2 * rr + 1] = 1.0
        jmat[2 * rr + 1, 2 * rr] = -1.0
    rotc = np.zeros((128, 32), np.float32)
    rotc[0:16, 0:16] = eye
    rotc[0:16, 16:32] = jmat
    rotc = np.ascontiguousarray(rotc.astype(ml_dtypes.bfloat16))
    mats = {
        "mt_s": np.asarray(inputs["raw_S"], np.float32),
        "mt_p": np.asarray(inputs["raw_phase"], np.float32),
        "mt_r": np.asarray(inputs["raw_r"], np.float32),
        "mt_m": np.asarray(inputs["A_mask"], np.float32),
        "mt_g": np.asarray(inputs["G_gate"], np.float32),
    }
    in_maps = []
    for ci in range(NCORES):
        rows = slice(ci * ROWS, (ci + 1) * ROWS)
        im = {}
        for k, v in mats.items():
            # [512m, 4096k] -> T -> [32kt, 128p, 512m] -> [128, 32*512]
            dt = ml_dtypes.float8_e4m3 if k in ("mt_m", "mt_g") else \
                ml_dtypes.bfloat16
            mt = v[rows].T.reshape(KT, 128, ROWS).transpose(1, 0, 2)
            im[k] = np.ascontiguousarray(
                mt.reshape(128, KT * ROWS).astype(dt)
            )
        im["xfull"] = xT
        im["wrot"] = wrot
        im["rotc"] = rotc
        in_maps.append(im)
    res = bass_utils.run_bass_kernel_spmd(
        nc, in_maps, core_ids=list(range(NCORES)), trace=trace
    )
    # reassemble: core i owns output columns [512*i, 512*(i+1))
    full = np.empty((STEPS + 1, 2, N), np.float32)
    full[0] = xT
    for i in range(NCORES):
        oi = np.asarray(res.results[i]["out"]).astype(np.float32)
        full[1:, :, i * ROWS:(i + 1) * ROWS] = oi[1:]
    return np.ascontiguousarray(full.transpose(0, 2, 1)), res


def kernel(**inputs):
    full, _ = run(inputs, trace=False)
    return full
